# revision 32
# baseline (speedup 1.0000x reference)
"""FNS spectral network kernel for 8x TRN2 NeuronCores (data parallel over batch).

v2: pair-blocked convs with batched evictions + staged halo zones.

Math (verified vs reference in fp32 sim to ~5e-4 incl fp16 weights):
  per sample b:
    rh = (-Gi) @ r @ Gi.T          Gi[j,n] = sin(pi*(j-128)*(n+1)/256)/256
    x  = conv1..3 -> *theta -> conv4..6   (3x3 per-sample complex convs)
    e  = H @ x @ H.T               H[k,j] = exp(-2i*pi*k*(j-127)/513)

Device mapping (1 sample/core):
  - Block-Toeplitz convs: image rows in 43 blocks of 6 (1-row halo each side),
    lanes = (row-in-block, reim*8+ch). X layer tiles are single 3D SBUF
    tiles [128, 43, 259] (259 = 257 data cols + zero pad col each side).
  - Two blocks share one PSUM bank: matmuls emit 256-col outputs
    ([128, 2, 256] psum tile = exactly 2KB/partition), the 257th output
    column of all 43 blocks is computed by a separate per-layer 3-matmul
    batch with a block-strided rhs.
  - Evictions are 2 ops per pair (512 cols each): mids [0:96] -> X tile,
    dup zones [96:128] -> staging tile z. Engines alternate per pair
    (even: ACT mids / DVE zones, odd: swapped) so each engine does one
    ~650ns op per ~660ns of PE work -- all three stay saturated.
  - Halo propagation: grouped 3D-strided SBUF DMAs (z -> next layer's
    il0/il7 partition zones), ~12 DMAs per layer on sync/gpsimd queues.
  - theta: conv3 emits two streams (t3/t3s re/im-swapped); DVE multiplies
    them by sign-baked resident theta tiles, GpSimd adds into x4.
  - Constants arrive in 4 fused DRAM params (few fat DMAs instead of ~30);
    theta (4.9MB) streams in 6 chunks during conv1/conv2.
  - fp16 operands, fp32 PSUM accumulate. Bacc for TRN2 legalization.
"""

import os

import numpy as np

import concourse.bacc as bacc
import concourse.mybir as mybir
from concourse.bass_utils import run_bass_kernel_spmd
from concourse.tile import TileContext

F16 = mybir.dt.float16
F32 = mybir.dt.float32

B = 8
N1 = 255
CROP = 257
CH = 8
NBLK = 43
SLOT = 259
NPAIR = 22          # 21 full pairs + block 42 alone

LAST_EXEC_TIME_NS = None

# tpk column offsets
_TORDER = ("t1", "t2", "t2b", "t3", "t3b", "t3s", "t3sb", "t4", "t4b",
           "t5", "t5b")
TPK_OFF = {k: i * 384 for i, k in enumerate(_TORDER)}
TPK_OFF["t6"] = 11 * 384
TPK_OFF["t6b"] = 11 * 384 + 36
TPK_COLS = 11 * 384 + 72


# ----------------------------------------------------------------------------
# Host-side constant / weight preprocessing
# ----------------------------------------------------------------------------

def _host_consts():
    j = np.arange(CROP)[:, None]
    n = np.arange(N1)[None, :]
    Gi = (np.sin(np.pi * (j - 128) * (n + 1) / 256.0) / 256.0).astype(np.float32)
    k = np.arange(N1)[:, None]
    jj = np.arange(CROP)[None, :]
    H = np.exp(-2j * np.pi * k * (jj - 127.0) / 513.0)
    return {
        "g1t": np.ascontiguousarray((-Gi).T.astype(np.float16)),   # [255,257]
        "g2t": np.ascontiguousarray(Gi.T.astype(np.float16)),      # [255,257]
        "hrt": np.ascontiguousarray(H.real.T.astype(np.float16)),  # [257,255]
        "hit": np.ascontiguousarray(H.imag.T.astype(np.float16)),
        "hnit": np.ascontiguousarray((-H.imag).T.astype(np.float16)),
    }


def _expand_w(wre, wim):
    """[Co,Ci,3,3] complex -> real packed [2Co, 2Ci, 3, 3], part = reim*C+ch."""
    Co, Ci = wre.shape[0], wre.shape[1]
    W = np.zeros((2 * Co, 2 * Ci, 3, 3), np.float32)
    W[:Co, :Ci] = wre
    W[:Co, Ci:] = -wim
    W[Co:, :Ci] = wim
    W[Co:, Ci:] = wre
    return W


def _wT(wre, wim):
    """torch _wT: swap cout/cin, transpose 3x3 kernel, conjugate."""
    wre2 = np.swapaxes(np.swapaxes(wre, 0, 1), -2, -1)
    wim2 = -np.swapaxes(np.swapaxes(wim, 0, 1), -2, -1)
    return wre2, wim2


def _row_std(p):
    if p < 96:
        return 1 + p // 16, p % 16
    if p < 112:
        return 0, p - 96
    return 7, p - 112


def _row_x4(p):
    """x4 (theta output) layout: re mid [0:48], il0 halos [48:64] (re,im),
    im mid [64:112], il7 halos [112:128] (re,im)."""
    if p < 48:
        return 1 + p // 8, p % 8
    if p < 64:
        return 0, p - 48
    if p < 112:
        q = p - 64
        return 1 + q // 8, 8 + q % 8
    return 7, p - 112


def _col_std_dup(m):
    if m < 96:
        return m // 16, m % 16
    if m < 112:
        return 5, m - 96
    return 0, m - 112


def _build_T(Wexp, rowmap, colmap, K, M, zero42=False):
    T = np.zeros((K, 3 * M), np.float32)
    Cin2 = Wexp.shape[1]
    for p in range(K):
        il, cp = rowmap(p)
        if cp >= Cin2:
            continue
        if zero42 and il >= 6:
            continue
        for dj in range(3):
            for m in range(M):
                co = colmap(m)
                if co is None:
                    continue
                inn, op = co
                di = il - inn
                if 0 <= di <= 2:
                    T[p, dj * M + m] = Wexp[op, cp, di, dj]
    return T.astype(np.float16)


def _host_prep_sample(bidx, inputs, consts):
    w1 = (inputs["w1_re"][bidx], inputs["w1_im"][bidx])
    w2 = (inputs["w2_re"][bidx], inputs["w2_im"][bidx])
    w3 = (inputs["w3_re"][bidx], inputs["w3_im"][bidx])

    W1r = _expand_w(*w1)[:, 0:1]
    W2 = _expand_w(*w2)
    W3 = _expand_w(*w3)
    W4 = _expand_w(*_wT(*w3))
    W5 = _expand_w(*_wT(*w2))
    W6 = _expand_w(*_wT(*w1))

    def col_c3a(m):
        if m < 48:
            return m // 8, m % 8
        if 64 <= m < 112:
            q = m - 64
            return q // 8, 8 + q % 8
        return None

    def col_c3b(m):
        if m < 48:
            return m // 8, 8 + m % 8
        if 64 <= m < 112:
            q = m - 64
            return q // 8, q % 8
        return None

    def col_c6(m):
        return m // 2, m % 2

    def row_x1(p):
        return p, 0

    tm = {
        "t1": _build_T(W1r, row_x1, _col_std_dup, 8, 128),
        "t2": _build_T(W2, _row_std, _col_std_dup, 128, 128),
        "t2b": _build_T(W2, _row_std, _col_std_dup, 128, 128, zero42=True),
        "t3": _build_T(W3, _row_std, col_c3a, 128, 128),
        "t3b": _build_T(W3, _row_std, col_c3a, 128, 128, zero42=True),
        "t3s": _build_T(W3, _row_std, col_c3b, 128, 128),
        "t3sb": _build_T(W3, _row_std, col_c3b, 128, 128, zero42=True),
        "t4": _build_T(W4, _row_x4, _col_std_dup, 128, 128),
        "t4b": _build_T(W4, _row_x4, _col_std_dup, 128, 128, zero42=True),
        "t5": _build_T(W5, _row_std, _col_std_dup, 128, 128),
        "t5b": _build_T(W5, _row_std, _col_std_dup, 128, 128, zero42=True),
        "t6": _build_T(W6, _row_std, col_c6, 128, 12),
        "t6b": _build_T(W6, _row_std, col_c6, 128, 12, zero42=True),
    }
    tpk = np.zeros((128, TPK_COLS), np.float16)
    for kk, t in tm.items():
        off = TPK_OFF[kk]
        tpk[0:t.shape[0], off:off + t.shape[1]] = t

    # theta: [112, NBLK * 514]; X part cols [b*514 : b*514+257] (main cols
    # j=0..255 then lastcol j=256), Y part at +257 with signs baked
    tr = inputs["theta_re"][bidx]
    ti = inputs["theta_im"][bidx]
    th = np.zeros((112, NBLK, 2 * CROP), np.float16)
    for bb in range(NBLK):
        ninn = 6 if bb < NBLK - 1 else 5
        for inn in range(ninn):
            row = 6 * bb + inn
            for ch in range(CH):
                p = inn * 8 + ch
                th[p, bb, 0:CROP] = tr[ch, row]
                th[64 + p, bb, 0:CROP] = tr[ch, row]
                th[p, bb, CROP:2 * CROP] = -ti[ch, row]
                th[64 + p, bb, CROP:2 * CROP] = ti[ch, row]

    r16 = inputs["r"][bidx, 0].astype(np.float16)
    rgg = np.concatenate([r16, consts["g1t"], consts["g2t"]], axis=1)
    hhh = np.concatenate([consts["hrt"], consts["hit"], consts["hnit"]],
                         axis=1)
    return {
        "rgg": np.ascontiguousarray(rgg),                    # [255, 769]
        "hhh": np.ascontiguousarray(hhh),                    # [257, 765]
        "tpk": np.ascontiguousarray(tpk),                    # [128, 4296]
        "thet": np.ascontiguousarray(th.reshape(112, NBLK * 2 * CROP)),
    }


# ----------------------------------------------------------------------------
# Device program
# ----------------------------------------------------------------------------

def _build_nc(dbg=False):
    nc = bacc.Bacc(None, target_bir_lowering=False, debug=False)

    dp = {}
    for name, shape in (("rgg", [N1, 769]), ("hhh", [CROP, 765]),
                        ("tpk", [128, TPK_COLS]),
                        ("thet", [112, NBLK * 2 * CROP])):
        dp[name] = nc.declare_dram_parameter(name, shape, F16, isOutput=False)
    ere = nc.declare_dram_parameter("ere", [N1, N1], F16, isOutput=True)
    eim = nc.declare_dram_parameter("eim", [N1, N1], F16, isOutput=True)
    dbg_out = {}
    if dbg:
        for nm in ("dx2", "dx3", "dx4", "dx5", "dx6"):
            dbg_out[nm] = nc.declare_dram_parameter(
                nm, [128, NBLK * SLOT], F16, isOutput=True)
        dbg_out["dy6"] = nc.declare_dram_parameter(
            "dy6", [12, NBLK * 257], F16, isOutput=True)


    with TileContext(nc) as tc:
        with (
            tc.tile_pool(name="const", bufs=1) as pc,
            tc.tile_pool(name="xbuf", bufs=1) as px,
            tc.tile_pool(name="work", bufs=1) as pw,
            tc.tile_pool(name="uv", bufs=3) as puv,
            tc.tile_pool(name="ppair", bufs=4, space="PSUM") as pp,
            tc.tile_pool(name="pwide", bufs=2, space="PSUM") as ppw,
            tc.tile_pool(name="plc", bufs=2, space="PSUM") as ppl,
        ):
            # ---------------- constant loads (few fat DMAs) ----------------
            rgg_sb = [pc.tile([128, 769], F16, name="rgg0", tag="rgg0"),
                      pc.tile([127, 769], F16, name="rgg1", tag="rgg1")]
            # 4 half-height DMAs ride 4 parallel rings (~2x faster wall
            # than 2 full-height ones under 8-core HBM contention)
            nc.sync.dma_start(rgg_sb[0][0:32, :], dp["rgg"][0:32, :])
            nc.sync.dma_start(rgg_sb[0][32:64, :], dp["rgg"][32:64, :])
            nc.sync.dma_start(rgg_sb[0][64:96, :], dp["rgg"][64:96, :])
            nc.sync.dma_start(rgg_sb[0][96:128, :], dp["rgg"][96:128, :])
            nc.sync.dma_start(rgg_sb[1][0:64, :], dp["rgg"][128:192, :])
            nc.sync.dma_start(rgg_sb[1][64:127, :], dp["rgg"][192:255, :])

            tpk = pc.tile([128, TPK_COLS], F16, name="tpk", tag="tpk")
            # progressive chunks: t1 first (tiny, unblocks conv1), then the
            # per-layer groups in consumption order
            nc.sync.dma_start(tpk[0:8, 0:384], dp["tpk"][0:8, 0:384])
            nc.gpsimd.dma_start(tpk[0:64, 384:1152],
                                dp["tpk"][0:64, 384:1152])
            nc.gpsimd.dma_start(tpk[64:128, 384:1152],
                                dp["tpk"][64:128, 384:1152])

            hhh_sb = [pc.tile([128, 765], F16, name="hhh0", tag="hhh0"),
                      pc.tile([128, 765], F16, name="hhh1", tag="hhh1"),
                      pc.tile([1, 765], F16, name="hhh2", tag="hhh2")]

            thet = pc.tile([112, NBLK, 2 * CROP], F16, name="thet", tag="thet")
            # loaded in 6 gated chunks after the front transform
            _TH_CHUNKS = [(0, 4), (4, 8), (8, 12), (12, 16), (16, 20),
                          (20, 24), (24, 28), (28, 32), (32, 36), (36, 39),
                          (39, 41), (41, 43)]

            def load_theta_chunk(i, eng):
                b0, b1 = _TH_CHUNKS[i]
                eng.dma_start(thet[:, b0:b1, :],
                              dp["thet"][:, b0 * 2 * CROP:b1 * 2 * CROP])

            # ---------------- X tiles + memsets ----------------
            Xa = px.tile([128, NBLK, SLOT], F16, name="Xa", tag="Xa")
            Xb = px.tile([128, NBLK, SLOT], F16, name="Xb", tag="Xb")
            Xc = px.tile([128, NBLK, SLOT], F16, name="Xc", tag="Xc")
            S = px.tile([12, NBLK, SLOT], F16, name="S", tag="S")
            z = px.tile([128, NBLK, CROP], F16, name="z", tag="z")

            for Xt in (Xa, Xb, Xc):
                nc.vector.memset(Xt[:, :, 0:1], 0.0)
                nc.vector.memset(Xt[:, :, 258:259], 0.0)
            nc.vector.memset(S[0:8, :, 0:1], 0.0)
            nc.vector.memset(S[0:8, :, 258:259], 0.0)
            nc.vector.memset(S[0:1, 0:1, :], 0.0)        # X1 row -1 (blk 0)
            nc.vector.memset(S[0:8, 42:43, :], 0.0)      # X1 blk42 (rows 257/258; valid rows rewritten by scatter)
            for Xt in (Xa, Xb):
                nc.vector.memset(Xt[96:112, 0:1, :], 0.0)    # il0 of blk 0
                nc.vector.memset(Xt[96:128, 42:43, :], 0.0)   # il7 of blk 42
            nc.vector.memset(Xc[96:128, 42:43, :], 0.0)

            # ---------------- front transform ----------------
            # Vt = (G1 r)^T ; rh = G1 r G2^T
            vt_sb = [pw.tile([128, CROP], F16, name="vt0", tag="vt0"),
                     pw.tile([127, CROP], F16, name="vt1", tag="vt1")]
            for m, (m0, mm) in enumerate(((0, 128), (128, 127))):
                ps = ppw.tile([128, CROP], F32, name="pw", tag="pw")
                for k2 in range(2):
                    nc.tensor.matmul(
                        ps[0:mm, :], lhsT=rgg_sb[k2][:, m0:m0 + mm],
                        rhs=rgg_sb[k2][:, 255:512], start=(k2 == 0),
                        stop=(k2 == 1))
                nc.scalar.copy(vt_sb[m][:, :], ps[0:mm, :])

            rh_sb = [pw.tile([128, CROP], F16, name="rh0", tag="rh0"),
                     pw.tile([128, CROP], F16, name="rh1", tag="rh1"),
                     pw.tile([1, CROP], F16, name="rh2", tag="rh2")]
            for m, (m0, mm) in enumerate(((0, 128), (128, 128), (256, 1))):
                ps = ppw.tile([128, CROP], F32, name="pw", tag="pw")
                for k2 in range(2):
                    nc.tensor.matmul(
                        ps[0:mm, :], lhsT=vt_sb[k2][:, m0:m0 + mm],
                        rhs=rgg_sb[k2][:, 512:769], start=(k2 == 0),
                        stop=(k2 == 1))
                nc.vector.tensor_copy(rh_sb[m][:, :], ps[0:mm, :])

            qrr = [nc.sync, nc.gpsimd]
            qi = 0
            for bb in range(NBLK):
                lo = max(0, 6 * bb - 1)
                hi = min(256, 6 * bb + 6)
                r0 = lo
                while r0 <= hi:
                    c = r0 // 128
                    c_end = min(hi, c * 128 + 127)
                    cnt = c_end - r0 + 1
                    il0 = r0 - (6 * bb - 1)
                    qrr[qi % 2].dma_start(
                        S[il0:il0 + cnt, bb:bb + 1, 1:258],
                        rh_sb[c][r0 - c * 128:r0 - c * 128 + cnt, 0:257])
                    qi += 1
                    r0 = c_end + 1

            # gated bulk loads: tiny DMAs reading rh_sb create a real RAW
            # dep on the front transform, so the big transfers cannot race
            # ahead of the critical rgg/tpk loads on the DMA rings.
            nc.sync.dma_start(thet[0:112, :, 0:1], rh_sb[0][0:112, 0:43])
            # gate the late T chunks too (conv3+ weights)
            nc.gpsimd.dma_start(tpk[0:1, 1152:4296:768],
                                rh_sb[0][0:1, 0:5])
            for c0, c1 in ((1152, 2688), (2688, 3456), (3456, 4224),
                           (4224, TPK_COLS)):
                nc.gpsimd.dma_start(tpk[:, c0:c1], dp["tpk"][:, c0:c1])
            for i in range(len(_TH_CHUNKS)):
                load_theta_chunk(i, nc.gpsimd)
            nc.gpsimd.dma_start(hhh_sb[0][:, 0:1], rh_sb[0][0:128, 0:1])
            nc.gpsimd.dma_start(hhh_sb[1][:, 0:1], rh_sb[0][0:128, 1:2])
            for c, (r0, rr) in enumerate(((0, 128), (128, 128), (256, 1))):
                nc.scalar.dma_start(hhh_sb[c][:, :], dp["hhh"][r0:r0 + rr, :])

            # PE warm-up during the X1 scatter (ramps DVFS before conv1)
            wps = pp.tile([128, 2, 256], F32, name="pp", tag="pp")
            for _ in range(16):
                nc.tensor.matmul(wps[0:128, 0, :],
                                 lhsT=rgg_sb[0][:, 0:128],
                                 rhs=rgg_sb[0][:, 0:256],
                                 start=True, stop=True)
            trash = pw.tile([1, 4], F16, name="trash", tag="trash")
            nc.vector.tensor_copy(trash[0:1, 0:2], wps[0:1, 0, 0:2])

            # ---------------- conv machinery ----------------
            # zone-propagation groups (fire once src-max block evicted)
            IL0_GROUPS = [(0, 7, 7), (8, 15, 15), (16, 23, 23),
                          (24, 31, 31), (32, 39, 39), (40, 41, 42)]
            IL7_GROUPS = [(1, 7), (8, 15), (16, 23), (24, 31), (32, 39),
                          (40, 42)]
            LC_GROUPS = [(0, 8), (8, 16), (16, 24), (24, 32), (32, 40),
                         (40, 43)]

            def lc_group_mms(psl, toff, toffb, has_b, Xin, K, g0, g1):
                hi = min(g1, 42) if has_b else g1
                for dj in range(3):
                    nc.tensor.matmul(
                        psl[0:128, g0:hi, :],
                        lhsT=tpk[0:K, toff + dj * 128:toff + (dj + 1) * 128],
                        rhs=Xin[0:K, g0:hi, 256 + dj:257 + dj],
                        start=(dj == 0), stop=(dj == 2))
                if has_b and g1 == 43:
                    for dj in range(3):
                        nc.tensor.matmul(
                            psl[0:128, 42:43, :],
                            lhsT=tpk[0:K, toffb + dj * 128:toffb + (dj + 1) * 128],
                            rhs=Xin[0:K, 42:43, 256 + dj:257 + dj],
                            start=(dj == 0), stop=(dj == 2))

            def conv_std(tname, Xin, Xout, K, post_pair=None):
                """One block-Toeplitz conv layer with pair-batched psum and
                per-group-of-8 lastcol batches (keeps layer boundaries off the
                critical path)."""
                has_b = tname != "t1"
                toff = TPK_OFF[tname]
                toffb = TPK_OFF[tname + "b"] if has_b else toff
                psl = ppl.tile([128, 48, 1], F32, name="plc", tag="plc")

                il0_next = 0
                il7_next = 0
                lc_next = 0
                qi2 = 0
                for p in range(NPAIR):
                    b0 = 2 * p
                    nbp = 1 if b0 == 42 else 2
                    ps = pp.tile([128, 2, 256], F32, name="pp", tag="pp")
                    for ki in range(nbp):
                        bb = b0 + ki
                        off = toffb if (has_b and bb == 42) else toff
                        for dj in range(3):
                            nc.tensor.matmul(
                                ps[0:128, ki, :],
                                lhsT=tpk[0:K, off + dj * 128:off + (dj + 1) * 128],
                                rhs=Xin[0:K, bb, dj:dj + 256],
                                start=(dj == 0), stop=(dj == 2))
                    e_mid = nc.scalar.copy if p % 2 == 0 else nc.vector.tensor_copy
                    e_zon = nc.vector.tensor_copy if p % 2 == 0 else nc.scalar.copy
                    e_mid(Xout[0:96, b0:b0 + nbp, 1:257], ps[0:96, 0:nbp, :])
                    e_zon(z[96:128, b0:b0 + nbp, 0:256], ps[96:128, 0:nbp, :])
                    bmax = b0 + nbp - 1
                    while lc_next < len(LC_GROUPS) and \
                            LC_GROUPS[lc_next][1] - 1 <= bmax:
                        g0, g1 = LC_GROUPS[lc_next]
                        lc_group_mms(psl, toff, toffb, has_b, Xin, K, g0, g1)
                        e_lc1 = nc.scalar.copy if lc_next % 2 else nc.vector.tensor_copy
                        e_lc2 = nc.vector.tensor_copy if lc_next % 2 else nc.scalar.copy
                        e_lc1(Xout[0:96, g0:g1, 257:258], psl[0:96, g0:g1, :])
                        e_lc2(z[96:128, g0:g1, 256:257], psl[96:128, g0:g1, :])
                        lc_next += 1
                    while il0_next < len(IL0_GROUPS) and \
                            IL0_GROUPS[il0_next][2] <= bmax:
                        a0, a1, _ = IL0_GROUPS[il0_next]
                        nc.sync.dma_start(
                            Xout[96:112, a0 + 1:a1 + 2, 1:258],
                            z[96:112, a0:a1 + 1, 0:257])
                        qi2 += 1
                        il0_next += 1
                    while il7_next < len(IL7_GROUPS) and \
                            IL7_GROUPS[il7_next][1] <= bmax:
                        a0, a1 = IL7_GROUPS[il7_next]
                        nc.sync.dma_start(
                            Xout[112:128, a0 - 1:a1, 1:258],
                            z[112:128, a0:a1 + 1, 0:257])
                        qi2 += 1
                        il7_next += 1
                    if post_pair is not None:
                        post_pair(p)

            conv_std("t1", S, Xa, 8)
            if dbg:
                nc.sync.dma_start(dbg_out["dx2"][:, :], Xa[:, :, :])
            conv_std("t2", Xa, Xb, 128)
            if dbg:
                nc.sync.dma_start(dbg_out["dx3"][:, :], Xb[:, :, :])

            # ---------------- conv3 + theta -> Xc ----------------
            pslA = ppl.tile([128, 48, 1], F32, name="plc", tag="plc")
            pslB = ppl.tile([128, 48, 1], F32, name="plc", tag="plc")
            ulc = pw.tile([112, 48, 1], F16, name="ulc", tag="ulc")
            vlc = pw.tile([112, 48, 1], F16, name="vlc", tag="vlc")

            X4_IL0 = [(0, 7), (8, 15), (16, 23), (24, 31), (32, 39), (40, 41)]
            X4_IL7 = [(1, 7), (8, 15), (16, 23), (24, 31), (32, 39), (40, 42)]
            il0n = [0]
            il7n = [0]
            lc3n = [0]
            for p in range(NPAIR):
                b0 = 2 * p
                nbp = 1 if b0 == 42 else 2
                psA = pp.tile([128, 2, 256], F32, name="pp", tag="pp")
                psB = pp.tile([128, 2, 256], F32, name="pp", tag="pp")
                for tn, ps in (("t3", psA), ("t3s", psB)):
                    for ki in range(nbp):
                        bb = b0 + ki
                        key = tn + "b" if bb == 42 else tn
                        off = TPK_OFF[key]
                        for dj in range(3):
                            nc.tensor.matmul(
                                ps[0:128, ki, :],
                                lhsT=tpk[:, off + dj * 128:off + (dj + 1) * 128],
                                rhs=Xb[:, bb, dj:dj + 256],
                                start=(dj == 0), stop=(dj == 2))
                u = puv.tile([112, 2, 256], F16, name="u", tag="u")
                v = puv.tile([112, 2, 256], F16, name="v", tag="v")
                nc.vector.tensor_mul(u[:, 0:nbp, :], psA[0:112, 0:nbp, :],
                                     thet[:, b0:b0 + nbp, 0:256])
                nc.vector.tensor_mul(v[:, 0:nbp, :], psB[0:112, 0:nbp, :],
                                     thet[:, b0:b0 + nbp, 257:513])
                nc.gpsimd.tensor_add(Xc[0:112, b0:b0 + nbp, 1:257],
                                     u[:, 0:nbp, :], v[:, 0:nbp, :])
                bmax = b0 + nbp - 1
                while lc3n[0] < len(LC_GROUPS) and \
                        LC_GROUPS[lc3n[0]][1] - 1 <= bmax:
                    g0, g1 = LC_GROUPS[lc3n[0]]
                    for tn, psl in (("t3", pslA), ("t3s", pslB)):
                        lc_group_mms(psl, TPK_OFF[tn], TPK_OFF[tn + "b"],
                                     True, Xb, 128, g0, g1)
                    nc.vector.tensor_mul(ulc[:, g0:g1, :],
                                         pslA[0:112, g0:g1, :],
                                         thet[:, g0:g1, 256:257])
                    nc.vector.tensor_mul(vlc[:, g0:g1, :],
                                         pslB[0:112, g0:g1, :],
                                         thet[:, g0:g1, 513:514])
                    nc.gpsimd.tensor_add(Xc[0:112, g0:g1, 257:258],
                                         ulc[:, g0:g1, :], vlc[:, g0:g1, :])
                    lc3n[0] += 1
                while il0n[0] < len(X4_IL0) and \
                        X4_IL0[il0n[0]][1] + 1 <= bmax:
                    a0, a1 = X4_IL0[il0n[0]]
                    nc.sync.dma_start(Xc[48:56, a0 + 1:a1 + 2, 1:258],
                                      Xc[40:48, a0:a1 + 1, 1:258])
                    nc.sync.dma_start(Xc[56:64, a0 + 1:a1 + 2, 1:258],
                                      Xc[104:112, a0:a1 + 1, 1:258])
                    il0n[0] += 1
                while il7n[0] < len(X4_IL7) and X4_IL7[il7n[0]][1] <= bmax:
                    a0, a1 = X4_IL7[il7n[0]]
                    nc.sync.dma_start(Xc[112:120, a0 - 1:a1, 1:258],
                                      Xc[0:8, a0:a1 + 1, 1:258])
                    nc.sync.dma_start(Xc[120:128, a0 - 1:a1, 1:258],
                                      Xc[64:72, a0:a1 + 1, 1:258])
                    il7n[0] += 1

            conv_std("t4", Xc, Xa, 128)
            if dbg:
                nc.sync.dma_start(dbg_out["dx5"][:, :], Xa[:, :, :])
            conv_std("t5", Xa, Xb, 128)
            if dbg:
                nc.sync.dma_start(dbg_out["dx6"][:, :], Xb[:, :, :])

            # ---------------- conv6 -> y6 (in S) -> xo scatter -----------
            # xo2 chunks hold re/im interleaved: [row-partition, pi, col]
            xo2 = [pw.tile([128, 2, CROP], F16, name="xo20", tag="xo20"),
                   pw.tile([128, 2, CROP], F16, name="xo21", tag="xo21"),
                   pw.tile([1, 2, CROP], F16, name="xo22", tag="xo22")]

            t6o = TPK_OFF["t6"]
            t6ob = TPK_OFF["t6b"]
            psl6 = ppl.tile([128, 48, 1], F32, name="plc", tag="plc")
            lc6n = [0]
            qi6 = [0]
            q3 = [nc.sync, nc.gpsimd, nc.scalar]

            def post6(bhi):
                while qi6[0] <= bhi:
                    bb = qi6[0]
                    nil = 6 if bb < NBLK - 1 else 5
                    r0 = 6 * bb
                    while r0 < 6 * bb + nil:
                        c = r0 // 128
                        c_end = min(6 * bb + nil - 1, c * 128 + 127)
                        cnt = c_end - r0 + 1
                        il0 = r0 - 6 * bb
                        q3[(bb + r0) % 3].dma_start(
                            xo2[c][r0 - c * 128:r0 - c * 128 + cnt, 0:2, 0:257],
                            S[2 * il0:2 * il0 + 2 * cnt, bb:bb + 1, 0:257])
                        r0 = c_end + 1
                    qi6[0] += 1

            for p in range(NPAIR):
                b0 = 2 * p
                nbp = 1 if b0 == 42 else 2
                ps = pp.tile([128, 2, 256], F32, name="pp", tag="pp")
                for ki in range(nbp):
                    bb = b0 + ki
                    off = t6ob if bb == 42 else t6o
                    for dj in range(3):
                        nc.tensor.matmul(
                            ps[0:12, ki, :],
                            lhsT=tpk[:, off + dj * 12:off + (dj + 1) * 12],
                            rhs=Xb[:, bb, dj:dj + 256],
                            start=(dj == 0), stop=(dj == 2))
                e6 = nc.scalar.copy if p % 2 == 0 else nc.vector.tensor_copy
                e6(S[0:12, b0:b0 + nbp, 0:256], ps[0:12, 0:nbp, :])
                bmax = b0 + nbp - 1
                while lc6n[0] < len(LC_GROUPS) and \
                        LC_GROUPS[lc6n[0]][1] - 1 <= bmax:
                    g0, g1 = LC_GROUPS[lc6n[0]]
                    hi6 = min(g1, 42)
                    for dj in range(3):
                        nc.tensor.matmul(
                            psl6[0:12, g0:hi6, :],
                            lhsT=tpk[:, t6o + dj * 12:t6o + (dj + 1) * 12],
                            rhs=Xb[:, g0:hi6, 256 + dj:257 + dj],
                            start=(dj == 0), stop=(dj == 2))
                    if g1 == 43:
                        for dj in range(3):
                            nc.tensor.matmul(
                                psl6[0:12, 42:43, :],
                                lhsT=tpk[:, t6ob + dj * 12:t6ob + (dj + 1) * 12],
                                rhs=Xb[:, 42:43, 256 + dj:257 + dj],
                                start=(dj == 0), stop=(dj == 2))
                    nc.scalar.copy(S[0:12, g0:g1, 256:257],
                                   psl6[0:12, g0:g1, :])
                    lc6n[0] += 1
                    post6(g1 - 1)

            # PE warm-up during the scatter tail (keeps DVFS ramped)
            wps2 = pp.tile([128, 2, 256], F32, name="pp", tag="pp")
            for _ in range(12):
                nc.tensor.matmul(wps2[0:128, 0, :],
                                 lhsT=hhh_sb[0][:, 0:128],
                                 rhs=hhh_sb[0][:, 0:256],
                                 start=True, stop=True)
            trash2 = pw.tile([1, 4], F16, name="trash2", tag="trash2")
            nc.vector.tensor_copy(trash2[0:1, 0:2], wps2[0:1, 0, 0:2])

            # ---------------- back transform ----------------
            at = {}
            for pi_n in ("re", "im"):
                at[pi_n] = [
                    pw.tile([128, 256], F16, name=f"at{pi_n}0", tag=f"at{pi_n}0"),
                    pw.tile([128, 256], F16, name=f"at{pi_n}1", tag=f"at{pi_n}1"),
                    pw.tile([1, 256], F16, name=f"at{pi_n}2", tag=f"at{pi_n}2")]
                for t in at[pi_n]:
                    nc.vector.memset(t[:, 255:256], 0.0)
            hr = [t[:, 0:255] for t in hhh_sb]
            hi = [t[:, 255:510] for t in hhh_sb]
            hn = [t[:, 510:765] for t in hhh_sb]
            # per-(m, plane) psum groups; matmuls emitted k2-major so the
            # k2=0 terms overlap the tail of the conv6 xo scatter
            ATM = ((0, 128), (128, 128), (256, 1))
            atdef = {"re": (("re", hr), ("im", hn)),
                     "im": (("re", hi), ("im", hr))}
            at_groups = [(m, pn) for m in range(3) for pn in ("re", "im")]
            at_ps = {}
            for gi, (m, pn) in enumerate(at_groups):
                mm = ATM[m][1]
                if gi < 4:
                    t = pp.tile([128, 2, 256], F32, name="pp", tag="pp")
                    at_ps[(m, pn)] = t[0:mm, 0, 0:N1]
                else:
                    t = ppw.tile([128, CROP], F32, name="pw", tag="pw")
                    at_ps[(m, pn)] = t[0:mm, 0:N1]
            for k2 in range(3):
                for (m, pn) in at_groups:
                    m0, mm = ATM[m]
                    for ti_, (xp, hsb) in enumerate(atdef[pn]):
                        nc.tensor.matmul(
                            at_ps[(m, pn)],
                            lhsT=xo2[k2][:, 0 if xp == "re" else 1,
                                         m0:m0 + mm],
                            rhs=hsb[k2],
                            start=(k2 == 0 and ti_ == 0),
                            stop=(k2 == 2 and ti_ == 1))
            for (m, pn) in at_groups:
                nc.scalar.copy(at[pn][m][:, 0:N1], at_ps[(m, pn)])

            e_sb = {}
            for pi_n in ("re", "im"):
                e_sb[pi_n] = [
                    pw.tile([128, N1], F16, name=f"e{pi_n}0", tag=f"e{pi_n}0"),
                    pw.tile([127, N1], F16, name=f"e{pi_n}1", tag=f"e{pi_n}1")]
            EM = ((0, 128), (128, 128))   # m2=1 padded; at col 255 is zero
            e_groups = [(m2, pn) for m2 in range(2) for pn in ("re", "im")]
            e_tile = {}
            for (m2, pn) in e_groups:
                e_tile[(m2, pn)] = pp.tile([128, 2, 256], F32, name="pp",
                                           tag="pp")
            for k2 in range(3):
                for (m2, pn) in e_groups:
                    m0, mm = EM[m2]
                    for ti_, (ap_, hsb) in enumerate(atdef[pn]):
                        nc.tensor.matmul(
                            e_tile[(m2, pn)][0:mm, 0, 0:N1],
                            lhsT=at[ap_][k2][:, m0:m0 + mm],
                            rhs=hsb[k2],
                            start=(k2 == 0 and ti_ == 0),
                            stop=(k2 == 2 and ti_ == 1))
            for (m2, pn) in e_groups:
                rows = 128 if m2 == 0 else 127
                nc.vector.tensor_copy(e_sb[pn][m2][:, :],
                                      e_tile[(m2, pn)][0:rows, 0, 0:N1])

            # half-height output DMAs ride parallel rings (same rule as the
            # rgg input split); all queues are idle at writeback time
            oq = [nc.sync, nc.gpsimd, nc.scalar]
            for oi, (pi_n, dram) in enumerate((("re", ere), ("im", eim))):
                oq[(2 * oi) % 3].dma_start(dram[0:64, :],
                                           e_sb[pi_n][0][0:64, :])
                oq[(2 * oi + 1) % 3].dma_start(dram[64:128, :],
                                               e_sb[pi_n][0][64:128, :])
                oq[(2 * oi + 2) % 3].dma_start(dram[128:192, :],
                                               e_sb[pi_n][1][0:64, :])
                oq[(2 * oi + 3) % 3].dma_start(dram[192:255, :],
                                               e_sb[pi_n][1][64:127, :])
            if dbg:
                nc.sync.dma_start(dbg_out["dy6"][:, :], S[0:12, :, 0:257])

    nc.finalize()
    return nc


_NC_CACHE = None


def _get_nc():
    global _NC_CACHE
    dbg = bool(os.environ.get("KDBG"))
    if _NC_CACHE is None:
        _NC_CACHE = _build_nc(dbg=dbg)
    return _NC_CACHE


def kernel(**inputs):
    global LAST_EXEC_TIME_NS
    inputs = {k: np.asarray(v) for k, v in inputs.items()}
    consts = _host_consts()
    in_maps = [_host_prep_sample(b, inputs, consts) for b in range(B)]
    nc = _get_nc()
    trace = bool(os.environ.get("BASS_TRACE"))
    res = run_bass_kernel_spmd(nc, in_maps, list(range(B)), trace=trace)
    LAST_EXEC_TIME_NS = res.exec_time_ns
    out = np.zeros((B, 1, N1, N1), np.complex64)
    for b in range(B):
        out[b, 0] = res.results[b]["ere"] + 1j * res.results[b]["eim"]
    return out
